# revision 29
# baseline (speedup 1.0000x reference)
"""DTNNStep graph-message-passing kernel for 8x Trainium2 NeuronCores (v3).

Strategy: distance_membership_i is sorted, so pairs are sharded by
destination-atom range (6250 atoms per core). Each core processes 50
variable-width "windows" (<=128 atoms each, chosen on host so every
window holds <= 2048 pairs), so each window is exactly TPW=16 pair
tiles of 128 and the instruction stream is identical across cores.

The per-pair gather afh[j] is restructured: the HOST pre-gathers raw
atom_features[j] per pair (pure data layout) into a sequential bf16
stream gaT [128 emb, pairs]; the device computes
afh^T = W_cf^T @ gaT + b_cf per 512-pair block with a constant
stationary operand. This removes the device-side indirect-DMA gather,
the afh table build phase, and the per-tile PE transpose of the
baseline. dist^T is padded to 112 partitions (divisible by 16) so its
DMA stream sprays across all 16 DMA engines.

Pipeline per 512-pair block (all bf16 matmuls, f32 PSUM):
  dh^T  = wdfe^T @ distT-block          (PE, stationary wdfe const)
  afh^T = W_cf^T @ gaT-block            (PE, stationary W_cf const)
  biased = afh^T + b_cf                 (DVE, per-partition scalar)
  fusedT = biased * dh^T                (DVE)
  S[p,f] = (iota[f] == i'[p]) per tile  (DVE, one-hot dest-atom)
  mp     = fusedT-tile^T @ W_fc         (PE, per tile)
  msgs   = tanh(mp)                     (Scalar)
  win   += S^T @ msgs                   (PE PSUM accumulate, per tile)
Window flush computes the self-interaction term from the own-atom
columns appended to the gaT stream and writes out = win - ii + af.
No collectives: each core owns a disjoint output slice.
"""

import sys

for _p in ("/opt/trn_rl_repo",):
    if _p not in sys.path:
        sys.path.insert(0, _p)

import numpy as np
import ml_dtypes
import concourse.bass as bass
import concourse.bacc as bacc
import concourse.tile as tile
from concourse import mybir
from concourse.bass_utils import run_bass_kernel_spmd

F32 = mybir.dt.float32
BF16 = mybir.dt.bfloat16
F8E4 = mybir.dt.float8e4
NPBF = ml_dtypes.bfloat16
NPF8 = ml_dtypes.float8_e4m3

P = 128
N_ATOMS = 50000
N_PAIRS = 800000
N_EMB = 128
NCORES = 8
APC = N_ATOMS // NCORES            # atoms per core: 6250
TPW = 16                           # pair tiles per window
CAP = TPW * P                      # pair capacity per window: 2048
NWIN = 50                          # windows per core
NBLK = TPW // 4                    # 4-tile (512-pair) blocks per window
DPAD = 112                         # dist rows: 100 + bias row + pad (16|112)
GW = CAP + P                       # gaT cols per window (pairs + own atoms)
C16W = 5 * P + 640                 # bf16 const pack width


def build_nc():
    nc = bacc.Bacc()

    distT = nc.declare_dram_parameter("distT", [DPAD, NWIN * CAP], BF16,
                                      isOutput=False)
    gaT_d = nc.declare_dram_parameter("gaT", [P, NWIN * GW], BF16,
                                      isOutput=False)
    S_d = nc.declare_dram_parameter("Sst", [P, NWIN * CAP], F8E4,
                                    isOutput=False)
    cp16_d = nc.declare_dram_parameter("cp16", [P, C16W], BF16, isOutput=False)
    cp32_d = nc.declare_dram_parameter("cp32", [P, 2], F32, isOutput=False)
    out_d = nc.declare_dram_parameter("out", [NWIN * P, P], F32, isOutput=True)

    AT = mybir.AluOpType
    Tanh = mybir.ActivationFunctionType.Tanh
    Ident = mybir.ActivationFunctionType.Identity

    with tile.TileContext(nc) as tc:
        with (
            tc.tile_pool(name="consts", bufs=1) as cpool,
            tc.tile_pool(name="dist", bufs=4) as dist_pool,
            tc.tile_pool(name="ga", bufs=4) as ga_pool,
            tc.tile_pool(name="biased", bufs=4) as b_pool,
            tc.tile_pool(name="fusedT", bufs=4) as f_pool,
            tc.tile_pool(name="sgen", bufs=3) as s_pool,
            tc.tile_pool(name="msgs", bufs=4) as m_pool,
            tc.tile_pool(name="flush", bufs=3) as fl_pool,
            tc.tile_pool(name="ps_dh", bufs=2, space="PSUM") as dh_ps,
            tc.tile_pool(name="ps_ah", bufs=2, space="PSUM") as ah_ps,
            tc.tile_pool(name="ps_m", bufs=2, space="PSUM") as m_ps,
            tc.tile_pool(name="ps_win", bufs=2, space="PSUM") as win_ps,
        ):
            fl_ps = m_ps
            cpk = cpool.tile([P, C16W], BF16)
            nc.sync.dma_start(cpk[:], cp16_d[:])
            wcf = cpk[:, 0:P]
            wfc = cpk[:, P:2 * P]
            wdfe = cpk[:DPAD, 2 * P:3 * P]
            negident = cpk[:, 3 * P:4 * P]
            ident = cpk[:, 4 * P:5 * P]
            bcfr = cpk[0:1, 5 * P:5 * P + P]
            ones = cpk[0:1, 5 * P + P:5 * P + 640]
            cpk32 = cpool.tile([P, 2], F32)
            nc.sync.dma_start(cpk32[:], cp32_d[:])
            bcf = cpk32[:, 0:1]
            bdf = cpk32[:, 1:2]
            for w in range(NWIN):
                dt = dist_pool.tile([DPAD, CAP], BF16)
                nc.sync.dma_start(dt[:], distT[:, w * CAP:(w + 1) * CAP])
                ga = ga_pool.tile([P, GW], BF16)
                nc.sync.dma_start(ga[:], gaT_d[:, w * GW:(w + 1) * GW])
                St = s_pool.tile([P, CAP], F8E4)
                nc.gpsimd.dma_start(St[:], S_d[:, w * CAP:(w + 1) * CAP])

                win = win_ps.tile([P, P], F32)
                # residual init: win = (ga_own)^T @ I = af rows of window
                nc.tensor.matmul(win[:], lhsT=ga[:, CAP:CAP + P], rhs=ident,
                                 start=True, stop=False)

                # prologue: first block's matmuls + flush afh so PE can
                # run ahead of the DVE/Scalar stages
                ah0 = ah_ps.tile([P, 512], F32, tag="ah")
                nc.tensor.matmul(ah0[:], lhsT=wcf, rhs=ga[:, 0:512],
                                 start=True, stop=True)
                dh0 = dh_ps.tile([P, 512], F32, tag="dh")
                nc.tensor.matmul(dh0[:], lhsT=wdfe, rhs=dt[:, 0:512],
                                 start=True, stop=True)
                aho = fl_ps.tile([P, P], F32, tag="mp")
                nc.tensor.matmul(aho[:], lhsT=wcf, rhs=ga[:, CAP:CAP + P],
                                 start=True, stop=True)
                ipre = fl_pool.tile([P, P], BF16, tag="ipre")
                nc.vector.tensor_scalar(out=ipre[:], in0=aho[:], scalar1=bcf,
                                        scalar2=bdf, op0=AT.add, op1=AT.mult)

                ah, dh = ah0, dh0
                for b in range(NBLK):
                    # issue next block's heavy matmuls first (pipelining)
                    if b + 1 < NBLK:
                        c1 = (b + 1) * 512
                        ahn = ah_ps.tile([P, 512], F32, tag="ah")
                        nc.tensor.matmul(ahn[:], lhsT=wcf,
                                         rhs=ga[:, c1:c1 + 512],
                                         start=True, stop=True)
                        dhn = dh_ps.tile([P, 512], F32, tag="dh")
                        nc.tensor.matmul(dhn[:], lhsT=wdfe,
                                         rhs=dt[:, c1:c1 + 512],
                                         start=True, stop=True)
                    biased = b_pool.tile([P, 512], BF16)
                    if b % 2 == 0:
                        nc.scalar.activation(biased[:], ah[:], Ident,
                                             bias=bcf)
                    else:
                        nc.vector.tensor_scalar(out=biased[:], in0=ah[:],
                                                scalar1=bcf, scalar2=None,
                                                op0=AT.add)
                    fusedT = f_pool.tile([P, 512], BF16)
                    nc.vector.tensor_tensor(fusedT[:], biased[:], dh[:],
                                            op=AT.mult)
                    mp = m_ps.tile([P, 512], F32, tag="mp")
                    for s in range(4):
                        nc.tensor.matmul(mp[:, s * P:(s + 1) * P],
                                         lhsT=fusedT[:, s * P:(s + 1) * P],
                                         rhs=wfc, start=True, stop=True)
                    msgs = m_pool.tile([P, 512], BF16)
                    nc.scalar.activation(msgs[:], mp[:], Tanh)
                    for s in range(4):
                        kk = b * 4 + s
                        nc.tensor.matmul(win[:],
                                         lhsT=St[:, kk * P:(kk + 1) * P],
                                         rhs=msgs[:, s * P:(s + 1) * P],
                                         start=False, stop=False)
                    if b + 1 < NBLK:
                        ah, dh = ahn, dhn

                # ---- window flush ----
                iips = fl_ps.tile([P, P], F32, tag="mp")
                nc.tensor.matmul(iips[:], lhsT=ipre[:], rhs=wfc,
                                 start=True, stop=True)
                ii = fl_pool.tile([P, P], BF16, tag="ii")
                nc.scalar.activation(ii[:], iips[:], Tanh)
                nc.tensor.matmul(win[:], lhsT=negident, rhs=ii[:],
                                 start=False, stop=True)
                res = fl_pool.tile([P, P], F32, tag="res")
                nc.scalar.copy(res[:], win[:])
                nc.sync.dma_start(out_d[w * P:(w + 1) * P, :], res[:])

    nc.compile()
    return nc


def host_prep(atom_features, distance, atom_membership,
              distance_membership_i, distance_membership_j,
              W_cf, W_df, W_fc, b_cf, b_df):
    """Pack per-core inputs. Returns (in_maps, outmaps) where outmaps[c]
    maps each core-local atom row to its padded out-tensor row."""
    af = np.asarray(atom_features, np.float32)
    i = np.asarray(distance_membership_i, np.int64)
    j = np.asarray(distance_membership_j, np.int64)
    dist_bf = np.asarray(distance, np.float32).astype(NPBF)
    af_bf = af.astype(NPBF)
    af_ext = np.concatenate([af_bf, np.zeros((1, P), NPBF)], axis=0)
    counts = np.bincount(i, minlength=N_ATOMS)

    wdfe = np.zeros((DPAD, P), np.float32)
    wdfe[:100] = np.asarray(W_df, np.float32)
    wdfe[100] = np.asarray(b_df, np.float32)
    cp16 = np.zeros((P, C16W), np.float32)
    cp16[:, 0:P] = np.asarray(W_cf, np.float32)
    cp16[:, P:2 * P] = np.asarray(W_fc, np.float32)
    cp16[:DPAD, 2 * P:3 * P] = wdfe
    cp16[:, 3 * P:4 * P] = -np.eye(P, dtype=np.float32)
    cp16[:, 4 * P:5 * P] = np.eye(P, dtype=np.float32)
    cp16[0, 5 * P:5 * P + P] = np.asarray(b_cf, np.float32)
    cp16[0, 5 * P + P:5 * P + 640] = 1.0
    cp32 = np.zeros((P, 2), np.float32)
    cp32[:, 0] = np.asarray(b_cf, np.float32)
    cp32[:, 1] = np.asarray(b_df, np.float32)
    shared = {"cp16": cp16.astype(NPBF), "cp32": cp32}

    in_maps = []
    outmaps = []
    for c in range(NCORES):
        a_lo, a_hi = c * APC, (c + 1) * APC
        cnt = counts[a_lo:a_hi]
        # greedy max-fill: window takes atoms while <=128 atoms & <=CAP pairs
        bounds = [0]
        pos = 0
        while pos < APC:
            take, s = 0, 0
            while take < P and pos + take < APC and \
                    s + cnt[pos + take] <= CAP:
                s += cnt[pos + take]
                take += 1
            assert take > 0, "single atom exceeds window capacity"
            pos += take
            bounds.append(pos)
        assert len(bounds) - 1 <= NWIN, f"needs {len(bounds)-1} windows"
        while len(bounds) < NWIN + 1:
            bounds.append(APC)
        bounds = np.asarray(bounds, np.int64) + a_lo
        pb = np.searchsorted(i, bounds)
        npair = pb[1:] - pb[:-1]
        natom = bounds[1:] - bounds[:-1]
        assert npair.max() <= CAP

        colmap = np.full((NWIN, CAP), -1, np.int64)
        jmap = np.full((NWIN, GW), N_ATOMS, np.int64)
        ipr = np.full((NWIN, CAP), -1.0, np.float32)
        for w in range(NWIN):
            n = int(npair[w])
            colmap[w, :n] = np.arange(pb[w], pb[w + 1])
            jmap[w, :n] = j[pb[w]:pb[w + 1]]
            jmap[w, CAP:CAP + natom[w]] = np.arange(bounds[w], bounds[w + 1])
            ipr[w, :n] = (i[pb[w]:pb[w + 1]] - bounds[w]).astype(np.float32)

        flat = colmap.reshape(-1)
        m = flat >= 0
        dT = np.zeros((NWIN * CAP, DPAD), NPBF)
        dT[m, :100] = dist_bf[flat[m]]
        dT[m, 100] = 1.0
        distT_c = np.ascontiguousarray(dT.T)

        gaT_c = np.ascontiguousarray(af_ext[jmap.reshape(-1)].T)

        # one-hot segment-select matrices, streamed: S[p_pair, f_atom]
        Sf = np.zeros((NWIN, TPW, P, P), NPF8)
        ipr3 = ipr.reshape(NWIN, TPW, P)
        wi, si, pi = np.nonzero(ipr3 >= 0)
        Sf[wi, si, pi, ipr3[wi, si, pi].astype(np.int64)] = 1.0
        S_c = np.ascontiguousarray(
            Sf.transpose(2, 0, 1, 3).reshape(P, NWIN * CAP))

        rowmap = np.full((NWIN, P), -1, np.int64)
        outmap = np.empty(APC, np.int64)
        for w in range(NWIN):
            na = int(natom[w])
            rowmap[w, :na] = np.arange(bounds[w], bounds[w + 1])
            outmap[bounds[w] - a_lo:bounds[w + 1] - a_lo] = \
                w * P + np.arange(na)
        mdict = {
            "distT": distT_c,
            "gaT": gaT_c,
            "Sst": S_c,
        }
        mdict.update(shared)
        in_maps.append(mdict)
        outmaps.append(outmap)
    return in_maps, outmaps


def unshard(results, outmaps):
    out = np.empty((N_ATOMS, N_EMB), np.float32)
    for c in range(NCORES):
        out[c * APC:(c + 1) * APC] = results[c]["out"][outmaps[c]]
    return out


_NC_CACHE = {}


def get_nc():
    if "nc" not in _NC_CACHE:
        _NC_CACHE["nc"] = build_nc()
    return _NC_CACHE["nc"]


def kernel(**inputs):
    in_maps, outmaps = host_prep(**inputs)
    nc = get_nc()
    res = run_bass_kernel_spmd(nc, in_maps, core_ids=list(range(NCORES)))
    return unshard(res.results, outmaps)


# revision 31
# speedup vs baseline: 1.0153x; 1.0153x over previous
"""DTNNStep graph-message-passing kernel for 8x Trainium2 NeuronCores.

Strategy: distance_membership_i is sorted, so pairs are sharded by
destination-atom range (6250 atoms per core). Each core processes 50
variable-width "windows" (<=128 atoms each, chosen on host so every
window holds <= 2048 pairs), so each window is exactly TPW=16 pair
tiles of 128 and the instruction stream is identical across cores.

The per-pair gather afh[j] is restructured: the HOST pre-gathers raw
atom_features[j] per pair (pure data layout) into a sequential bf16
stream gaT [128 emb, pairs]; the device computes
afh^T = W_cf^T @ gaT + b_cf per 512-pair block with a constant
stationary operand. This removes the device-side indirect-DMA gather
(the 994ns-per-instruction SWDGE serialization) and the per-tile PE
transpose. dist^T is padded to 112 partitions (divisible by 16) so
its DMA stream sprays across all 16 DMA engines; the one-hot
segment-select matrices S are host-built and streamed as fp8 (exact
for 0/1), keeping all three compute engines off S generation.

Pipeline per 512-pair block (bf16 matmuls, f32 PSUM):
  dh^T   = wdfe^T @ distT-block      (PE, stationary wdfe const)
  afh^T  = W_cf^T @ gaT-block        (PE, stationary W_cf const)
  biased = afh^T + b_cf -> bf16 SBUF (Scalar/DVE alternating; also
                                      the required PSUM->SBUF stage)
  fusedT = biased * dh^T             (DVE)
  mp     = fusedT-tile^T @ W_fc      (PE, per tile)
  msgs   = tanh(mp)                  (Scalar)
  win   += S-tile^T @ msgs           (PE PSUM accumulate, per tile)
The window PSUM is initialized with the residual (ga_own^T @ I) and
finalized with -I @ ii (self-interaction), so the flush is just a
PSUM->SBUF copy + DMA. No collectives: each core owns a disjoint
output slice. Engine balance: PE/DVE/Scalar each ~77% active.
"""

import sys

for _p in ("/opt/trn_rl_repo",):
    if _p not in sys.path:
        sys.path.insert(0, _p)

import numpy as np
import ml_dtypes
import concourse.bass as bass
import concourse.bacc as bacc
import concourse.tile as tile
from concourse import mybir
from concourse.bass_utils import run_bass_kernel_spmd

F32 = mybir.dt.float32
BF16 = mybir.dt.bfloat16
F8E4 = mybir.dt.float8e4
NPBF = ml_dtypes.bfloat16
NPF8 = ml_dtypes.float8_e4m3

P = 128
N_ATOMS = 50000
N_PAIRS = 800000
N_EMB = 128
NCORES = 8
APC = N_ATOMS // NCORES            # atoms per core: 6250
TPW = 16                           # pair tiles per window
CAP = TPW * P                      # pair capacity per window: 2048
NWIN = 50                          # windows per core
NBLK = TPW // 4                    # 4-tile (512-pair) blocks per window
DPAD = 112                         # dist rows: 100 + bias row + pad (16|112)
GW = CAP + P                       # gaT cols per window (pairs + own atoms)
C16W = 5 * P + 640                 # bf16 const pack width


def build_nc():
    nc = bacc.Bacc()

    distT = nc.declare_dram_parameter("distT", [DPAD, NWIN * CAP], BF16,
                                      isOutput=False)
    gaT_d = nc.declare_dram_parameter("gaT", [P, NWIN * GW], BF16,
                                      isOutput=False)
    S_d = nc.declare_dram_parameter("Sst", [P, NWIN * CAP], F8E4,
                                    isOutput=False)
    cp16_d = nc.declare_dram_parameter("cp16", [P, C16W], BF16, isOutput=False)
    cp32_d = nc.declare_dram_parameter("cp32", [P, 2], F32, isOutput=False)
    out_d = nc.declare_dram_parameter("out", [NWIN * P, P], F32, isOutput=True)

    AT = mybir.AluOpType
    Tanh = mybir.ActivationFunctionType.Tanh
    Ident = mybir.ActivationFunctionType.Identity

    with tile.TileContext(nc) as tc:
        with (
            tc.tile_pool(name="consts", bufs=1) as cpool,
            tc.tile_pool(name="dist", bufs=3) as dist_pool,
            tc.tile_pool(name="ga", bufs=3) as ga_pool,
            tc.tile_pool(name="biased", bufs=3) as b_pool,
            tc.tile_pool(name="fusedT", bufs=3) as f_pool,
            tc.tile_pool(name="sgen", bufs=2) as s_pool,
            tc.tile_pool(name="msgs", bufs=3) as m_pool,
            tc.tile_pool(name="flush", bufs=2) as fl_pool,
            tc.tile_pool(name="ps_dh", bufs=2, space="PSUM") as dh_ps,
            tc.tile_pool(name="ps_ah", bufs=2, space="PSUM") as ah_ps,
            tc.tile_pool(name="ps_m", bufs=2, space="PSUM") as m_ps,
            tc.tile_pool(name="ps_win", bufs=2, space="PSUM") as win_ps,
        ):
            fl_ps = m_ps
            cpk = cpool.tile([P, C16W], BF16)
            nc.sync.dma_start(cpk[:], cp16_d[:])
            wcf = cpk[:, 0:P]
            wfc = cpk[:, P:2 * P]
            wdfe = cpk[:DPAD, 2 * P:3 * P]
            negident = cpk[:, 3 * P:4 * P]
            ident = cpk[:, 4 * P:5 * P]
            bcfr = cpk[0:1, 5 * P:5 * P + P]
            ones = cpk[0:1, 5 * P + P:5 * P + 640]
            cpk32 = cpool.tile([P, 2], F32)
            nc.sync.dma_start(cpk32[:], cp32_d[:])
            bcf = cpk32[:, 0:1]
            bdf = cpk32[:, 1:2]
            for w in range(NWIN):
                dt = dist_pool.tile([DPAD, CAP], BF16)
                nc.sync.dma_start(dt[:], distT[:, w * CAP:(w + 1) * CAP])
                ga = ga_pool.tile([P, GW], BF16)
                nc.sync.dma_start(ga[:], gaT_d[:, w * GW:(w + 1) * GW])
                St = s_pool.tile([P, CAP], F8E4)
                nc.gpsimd.dma_start(St[:], S_d[:, w * CAP:(w + 1) * CAP])

                win = win_ps.tile([P, P], F32)
                # residual init: win = (ga_own)^T @ I = af rows of window
                nc.tensor.matmul(win[:], lhsT=ga[:, CAP:CAP + P], rhs=ident,
                                 start=True, stop=False)

                # prologue: first block's matmuls + flush afh so PE can
                # run ahead of the DVE/Scalar stages
                ah0 = ah_ps.tile([P, 512], F32, tag="ah")
                nc.tensor.matmul(ah0[:], lhsT=wcf, rhs=ga[:, 0:512],
                                 start=True, stop=True)
                dh0 = dh_ps.tile([P, 512], F32, tag="dh")
                nc.tensor.matmul(dh0[:], lhsT=wdfe, rhs=dt[:, 0:512],
                                 start=True, stop=True)
                aho = fl_ps.tile([P, P], F32, tag="mp")
                nc.tensor.matmul(aho[:], lhsT=wcf, rhs=ga[:, CAP:CAP + P],
                                 start=True, stop=True)
                ipre = fl_pool.tile([P, P], BF16, tag="ipre")
                nc.vector.tensor_scalar(out=ipre[:], in0=aho[:], scalar1=bcf,
                                        scalar2=bdf, op0=AT.add, op1=AT.mult)

                ah, dh = ah0, dh0
                for b in range(NBLK):
                    # issue next block's heavy matmuls first (pipelining)
                    if b + 1 < NBLK:
                        c1 = (b + 1) * 512
                        ahn = ah_ps.tile([P, 512], F32, tag="ah")
                        nc.tensor.matmul(ahn[:], lhsT=wcf,
                                         rhs=ga[:, c1:c1 + 512],
                                         start=True, stop=True)
                        dhn = dh_ps.tile([P, 512], F32, tag="dh")
                        nc.tensor.matmul(dhn[:], lhsT=wdfe,
                                         rhs=dt[:, c1:c1 + 512],
                                         start=True, stop=True)
                    biased = b_pool.tile([P, 512], BF16)
                    if b % 2 == 0:
                        nc.scalar.activation(biased[:], ah[:], Ident,
                                             bias=bcf)
                    else:
                        nc.vector.tensor_scalar(out=biased[:], in0=ah[:],
                                                scalar1=bcf, scalar2=None,
                                                op0=AT.add)
                    fusedT = f_pool.tile([P, 512], BF16)
                    nc.vector.tensor_tensor(fusedT[:], biased[:], dh[:],
                                            op=AT.mult)
                    mp = m_ps.tile([P, 512], F32, tag="mp")
                    for s in range(4):
                        nc.tensor.matmul(mp[:, s * P:(s + 1) * P],
                                         lhsT=fusedT[:, s * P:(s + 1) * P],
                                         rhs=wfc, start=True, stop=True)
                    msgs = m_pool.tile([P, 512], BF16)
                    nc.scalar.activation(msgs[:], mp[:], Tanh)
                    for s in range(4):
                        kk = b * 4 + s
                        nc.tensor.matmul(win[:],
                                         lhsT=St[:, kk * P:(kk + 1) * P],
                                         rhs=msgs[:, s * P:(s + 1) * P],
                                         start=False, stop=False)
                    if b + 1 < NBLK:
                        ah, dh = ahn, dhn

                # ---- window flush ----
                iips = fl_ps.tile([P, P], F32, tag="mp")
                nc.tensor.matmul(iips[:], lhsT=ipre[:], rhs=wfc,
                                 start=True, stop=True)
                ii = fl_pool.tile([P, P], BF16, tag="ii")
                nc.scalar.activation(ii[:], iips[:], Tanh)
                nc.tensor.matmul(win[:], lhsT=negident, rhs=ii[:],
                                 start=False, stop=True)
                res = fl_pool.tile([P, P], F32, tag="res")
                nc.scalar.copy(res[:], win[:])
                nc.sync.dma_start(out_d[w * P:(w + 1) * P, :], res[:])

    nc.compile()
    return nc


def host_prep(atom_features, distance, atom_membership,
              distance_membership_i, distance_membership_j,
              W_cf, W_df, W_fc, b_cf, b_df):
    """Pack per-core inputs. Returns (in_maps, outmaps) where outmaps[c]
    maps each core-local atom row to its padded out-tensor row."""
    af = np.asarray(atom_features, np.float32)
    i = np.asarray(distance_membership_i, np.int64)
    j = np.asarray(distance_membership_j, np.int64)
    dist_bf = np.asarray(distance, np.float32).astype(NPBF)
    af_bf = af.astype(NPBF)
    af_ext = np.concatenate([af_bf, np.zeros((1, P), NPBF)], axis=0)
    counts = np.bincount(i, minlength=N_ATOMS)

    wdfe = np.zeros((DPAD, P), np.float32)
    wdfe[:100] = np.asarray(W_df, np.float32)
    wdfe[100] = np.asarray(b_df, np.float32)
    cp16 = np.zeros((P, C16W), np.float32)
    cp16[:, 0:P] = np.asarray(W_cf, np.float32)
    cp16[:, P:2 * P] = np.asarray(W_fc, np.float32)
    cp16[:DPAD, 2 * P:3 * P] = wdfe
    cp16[:, 3 * P:4 * P] = -np.eye(P, dtype=np.float32)
    cp16[:, 4 * P:5 * P] = np.eye(P, dtype=np.float32)
    cp16[0, 5 * P:5 * P + P] = np.asarray(b_cf, np.float32)
    cp16[0, 5 * P + P:5 * P + 640] = 1.0
    cp32 = np.zeros((P, 2), np.float32)
    cp32[:, 0] = np.asarray(b_cf, np.float32)
    cp32[:, 1] = np.asarray(b_df, np.float32)
    shared = {"cp16": cp16.astype(NPBF), "cp32": cp32}

    in_maps = []
    outmaps = []
    for c in range(NCORES):
        a_lo, a_hi = c * APC, (c + 1) * APC
        cnt = counts[a_lo:a_hi]
        # greedy max-fill: window takes atoms while <=128 atoms & <=CAP pairs
        bounds = [0]
        pos = 0
        while pos < APC:
            take, s = 0, 0
            while take < P and pos + take < APC and \
                    s + cnt[pos + take] <= CAP:
                s += cnt[pos + take]
                take += 1
            assert take > 0, "single atom exceeds window capacity"
            pos += take
            bounds.append(pos)
        assert len(bounds) - 1 <= NWIN, f"needs {len(bounds)-1} windows"
        while len(bounds) < NWIN + 1:
            bounds.append(APC)
        bounds = np.asarray(bounds, np.int64) + a_lo
        pb = np.searchsorted(i, bounds)
        npair = pb[1:] - pb[:-1]
        natom = bounds[1:] - bounds[:-1]
        assert npair.max() <= CAP

        colmap = np.full((NWIN, CAP), -1, np.int64)
        jmap = np.full((NWIN, GW), N_ATOMS, np.int64)
        ipr = np.full((NWIN, CAP), -1.0, np.float32)
        for w in range(NWIN):
            n = int(npair[w])
            colmap[w, :n] = np.arange(pb[w], pb[w + 1])
            jmap[w, :n] = j[pb[w]:pb[w + 1]]
            jmap[w, CAP:CAP + natom[w]] = np.arange(bounds[w], bounds[w + 1])
            ipr[w, :n] = (i[pb[w]:pb[w + 1]] - bounds[w]).astype(np.float32)

        flat = colmap.reshape(-1)
        m = flat >= 0
        dT = np.zeros((NWIN * CAP, DPAD), NPBF)
        dT[m, :100] = dist_bf[flat[m]]
        dT[m, 100] = 1.0
        distT_c = np.ascontiguousarray(dT.T)

        gaT_c = np.ascontiguousarray(af_ext[jmap.reshape(-1)].T)

        # one-hot segment-select matrices, streamed: S[p_pair, f_atom]
        Sf = np.zeros((NWIN, TPW, P, P), NPF8)
        ipr3 = ipr.reshape(NWIN, TPW, P)
        wi, si, pi = np.nonzero(ipr3 >= 0)
        Sf[wi, si, pi, ipr3[wi, si, pi].astype(np.int64)] = 1.0
        S_c = np.ascontiguousarray(
            Sf.transpose(2, 0, 1, 3).reshape(P, NWIN * CAP))

        rowmap = np.full((NWIN, P), -1, np.int64)
        outmap = np.empty(APC, np.int64)
        for w in range(NWIN):
            na = int(natom[w])
            rowmap[w, :na] = np.arange(bounds[w], bounds[w + 1])
            outmap[bounds[w] - a_lo:bounds[w + 1] - a_lo] = \
                w * P + np.arange(na)
        mdict = {
            "distT": distT_c,
            "gaT": gaT_c,
            "Sst": S_c,
        }
        mdict.update(shared)
        in_maps.append(mdict)
        outmaps.append(outmap)
    return in_maps, outmaps


def unshard(results, outmaps):
    out = np.empty((N_ATOMS, N_EMB), np.float32)
    for c in range(NCORES):
        out[c * APC:(c + 1) * APC] = results[c]["out"][outmaps[c]]
    return out


_NC_CACHE = {}


def get_nc():
    if "nc" not in _NC_CACHE:
        _NC_CACHE["nc"] = build_nc()
    return _NC_CACHE["nc"]


def kernel(**inputs):
    in_maps, outmaps = host_prep(**inputs)
    nc = get_nc()
    res = run_bass_kernel_spmd(nc, in_maps, core_ids=list(range(NCORES)))
    return unshard(res.results, outmaps)
